# revision 40
# baseline (speedup 1.0000x reference)
"""Trainium2 Bass kernel for a pre-norm transformer block (attention + MLP).

Problem: x:[2, 2048, 1024], 16 heads x 64, MLP hidden 4096, fp32.

Sharding: data parallel over tokens, ZERO collectives. The 4096 tokens are
split into 8 blocks of 512 (core c handles batch c//4, sequence block c%4).
Instead of all-gathering K/V, every core receives its batch's FULL 2048
tokens (permuted so its own 512 come first -- softmax is key-permutation
invariant) and redundantly computes LN1 + K/V projections for all of them.
That costs ~55us of extra (fp8) matmul per core but removes two AllGathers
(~135us measured) and all the DRAM round-trips around them. Each core then
runs attention for its own 512 queries over all 2048 keys and the MLP for
its own tokens. The host reassembles [2, 2048, 1024] from the 8 per-core
[512, 1024] blocks.

Numerics / layout strategy (per core):
  - LayerNorm affines folded into weights on the host (exact): ln_w scales
    W rows; ln1_b maps to a q bias (k bias cancels in softmax, v bias folds
    into proj_b), ln2_b folds into fc1_b.
  - LN1 stats run on a bf16 copy of x provided by the host (halves the DMA,
    doubles DVE throughput); the fp32 x rows are loaded only for the
    residual stream of the core's own 512 tokens. The normalize runs on the
    Activation engine (Identity with per-token scale=rstd, bias=-mu*rstd).
  - GEMMs with bf16/fp8 operands, fp32 PSUM. qkv / output-projection run
    fp8 (weights converted on the host); fc1/fc2 stay bf16 (fp8 there
    pushes error past the gate, even with scaled quantization).
  - Attention internals run in fp8e4: q^T, k^T, v and the probabilities.
    Scores are computed transposed, S^T[m, t] = k^T.T @ q^T with keys on
    partitions and the head dim (64) on contraction partitions.
    Probabilities use exp(S*scale - 4): the shift keeps exp() under fp8e4
    max (448) for this distribution (|S*scale| < ~8); the softmax
    normalization cancels the shift exactly, and the denominator is
    accumulated from the SAME fp8-quantized probabilities (ones column
    appended to v), so softmax stays exactly normalized. Scores land in
    bf16 PSUM tiles covering two key chunks, so each exp instruction
    processes 2048 elements per partition.
  - The attention-value matmuls use fp8 DoubleRow perf mode: two 128-key
    chunks contract per instruction at 0.5 cycles/row (2x bf16).
  - The output projection is folded into the attention pair loop: each
    finished head-pair immediately contributes its proj matmuls into two
    rotating PSUM banks, drained into the fp32 residual accumulator by the
    vector engine. This fills the tensor engine while the (activation-
    engine-bound) softmax exp stream paces the loop.
  - PE transposes run in bf16 (1 cycle/row vs 2 for fp32).
"""

import numpy as np
from contextlib import ExitStack

import concourse.bass as bass
import concourse.tile as tile
from concourse import mybir
from concourse.bass_utils import run_bass_kernel_spmd
from concourse.masks import make_identity

FP32 = mybir.dt.float32
FP32R = mybir.dt.float32r
BF16 = mybir.dt.bfloat16
F8 = mybir.dt.float8e4
AF = mybir.ActivationFunctionType
ALU = mybir.AluOpType
DR = mybir.MatmulPerfMode.DoubleRow

N_CORES = 8
B, N, C, H, D, F = 2, 2048, 1024, 16, 64, 4096
T = 512            # tokens owned per core
M = 2048           # keys (full batch sequence)
EPS = 1e-5
SCALE = float(D) ** -0.5   # 0.125
ESHIFT = -4.0      # exp(S*SCALE + ESHIFT): fp8-safe range for this dist

CB = C // 128      # 8 channel blocks
TB = T // 128      # 4 own-token blocks
MI = M // 128      # 16 key 128-chunks
MT = M // 128      # 16 full-batch token tiles
FB = F // 128      # 32 mlp hidden blocks

STOP_AFTER = ""    # debug: stop emission after a phase name
SKIP_CC = False    # unused (no collectives in this design); kept for tools


def _ln_group(nc, work, tiles, eps_sb):
    """Batched LN stats for a group of token tiles.

    tiles: list of (x_ap, out_xn_ap). Stats (bn_stats/bn_aggr) on DVE per
    tile; sqrt/reciprocal/-mu*rstd batched over the group; normalize on the
    Activation engine (Identity, per-partition scale/bias APs).
    """
    g = len(tiles)
    mv = work.tile([128, g, 2], FP32, name="ln_mv")
    for j, (x_ap, _) in enumerate(tiles):
        stats = work.tile([128, 2, 6], FP32, name="ln_stats")
        nc.vector.bn_stats(out=stats[:, 0, :], in_=x_ap[:, 0:512])
        nc.vector.bn_stats(out=stats[:, 1, :], in_=x_ap[:, 512:1024])
        nc.vector.bn_aggr(out=mv[:, j, :], in_=stats)
    sd = work.tile([128, g], FP32, name="ln_sd")
    nc.scalar.activation(out=sd, in_=mv[:, :, 1], func=AF.Sqrt,
                         bias=eps_sb, scale=1.0)
    rsig = work.tile([128, g], FP32, name="ln_rsig")
    nc.vector.reciprocal(out=rsig, in_=sd)
    nmr0 = work.tile([128, g], FP32, name="ln_nmr0")
    nc.vector.tensor_tensor(out=nmr0, in0=mv[:, :, 0], in1=rsig, op=ALU.mult)
    nmr = work.tile([128, g], FP32, name="ln_nmr")
    nc.vector.tensor_scalar(out=nmr, in0=nmr0, scalar1=-1.0, scalar2=None,
                            op0=ALU.mult)
    for j, (x_ap, xn_ap) in enumerate(tiles):
        nc.scalar.activation(out=xn_ap, in_=x_ap, func=AF.Identity,
                             scale=rsig[:, j:j + 1], bias=nmr[:, j:j + 1])


def _emit(ctx: ExitStack, tc: tile.TileContext, io: dict):
    nc = tc.nc

    xfull = io["xfull"]    # [2048, 1024] fp32 own-first permuted batch rows
    xbf = io["xbf"]        # [2048, 1024] bf16 copy of the same
    qkv_w = io["qkv_w"]    # [1024, 3072] (ln1_w folded in)
    proj_w = io["proj_w"]  # [1024, 1024]
    proj_b = io["proj_b"]  # [1024] (+ folded v bias)
    q_bias = io["q_bias"]  # [1024] folded ln1_b @ Wq
    fc1_w, fc1_b = io["fc1_w"], io["fc1_b"]   # ln2 folded in
    fc2_w, fc2_b = io["fc2_w"], io["fc2_b"]
    y = io["y"]            # [512, 1024] output

    xown_r = xfull.rearrange("(tb p) c -> p tb c", p=128)     # [128, 16, 1024]
    xbf_r = xbf.rearrange("(g p) c -> p g c", p=128)          # [128, 16, 1024]
    qkv_r = qkv_w.rearrange("(cb p) o -> p cb o", p=128)      # [128, 8, 3072]
    proj_r = proj_w.rearrange("(cb p) o -> p cb o", p=128)    # [128, 8, 1024]
    fc1_r = fc1_w.rearrange("(cb p) f -> p cb f", p=128)      # [128, 8, 4096]
    fc2_r = fc2_w.rearrange("(fb p) c -> p fb c", p=128)      # [128, 32, 1024]
    y_r = y.rearrange("(tb p) c -> p tb c", p=128)            # [128, 4, 1024]

    # --- constants (live whole kernel) ---
    consts = ctx.enter_context(tc.tile_pool(name="consts", bufs=1))

    ident_f = consts.tile([128, 128], FP32)
    make_identity(nc, ident_f)
    ident = consts.tile([128, 128], BF16)
    nc.vector.tensor_copy(out=ident, in_=ident_f)
    ones_b = consts.tile([128, 128], BF16)
    nc.vector.memset(ones_b, 1.0)
    eps_sb = consts.tile([128, 1], FP32)
    nc.vector.memset(eps_sb, EPS)
    eshift_sb = consts.tile([128, 1], FP32)
    nc.vector.memset(eshift_sb, ESHIFT)

    def load_vec_pcb(vec, nblk, name):
        t = consts.tile([128, nblk], FP32, name=name)
        nc.sync.dma_start(out=t, in_=vec.rearrange("(b p) -> p b", p=128))
        return t

    qb_sb = load_vec_pcb(q_bias, CB, "qb")
    fc1b_sb = load_vec_pcb(fc1_b, FB, "fc1b")

    def bcast_rows_pool(pool, vec, name):
        t = pool.tile([128, C], FP32, name=name)
        src = bass.AP(tensor=vec.tensor, offset=vec.offset, ap=[[0, 128]] + vec.ap)
        nc.sync.dma_start(out=t, in_=src)
        return t

    # --- persistent activations ---
    p_res = ctx.enter_context(tc.tile_pool(name="p_res", bufs=1))
    x2 = p_res.tile([128, TB, C], FP32)      # residual stream (starts as x)
    h2T = p_res.tile([128, CB, T], BF16)     # LN2 output, channel-major
    oT = p_res.tile([128, CB, T], F8)        # normalized attention out ^T

    # q^T / k^T packed for DoubleRow scores: partition = d%32, free =
    # (pair, head, d-half, t). Only partitions 0:32 are used. These live from
    # the projections through the pair loop, then free for the MLP weights.
    attn_ctx = ExitStack()
    p_attn = attn_ctx.enter_context(tc.tile_pool(name="p_attn", bufs=1))
    qTd = p_attn.tile([32, CB, 2, 2, T], F8)
    kTd = p_attn.tile([32, 4 * CB, M], F8, name="kTd")
    # v token-major per 128-key chunk: [key%128, chunk, head, d+1];
    # the softmax-denominator ones column lives at d=64.
    vg = p_attn.tile([128, MI, H, D + 1], F8, name="vga")

    # ---------------------------------------------------------------
    # Phase 1: LN1 over the FULL batch -> hT [c, t] fp8 channel-major
    # Phase 2: k/v projections (all 2048 keys); q projection (own 512)
    # ---------------------------------------------------------------
    with tc.tile_pool(name="p_h", bufs=1) as p_h:
        hT = p_h.tile([128, CB, M], F8)
        with (
            tc.tile_pool(name="ln1_in", bufs=2) as p_xin,
            tc.tile_pool(name="ln1_work", bufs=2) as w1,
            tc.tile_pool(name="ln1_ps", bufs=8, space="PSUM") as ps_t,
        ):
            for grp in range(4):
                xin = p_xin.tile([128, 4, C], BF16, name="ln1_xin")
                for half in range(2):
                    nc.sync.dma_start(
                        out=xin[:, half * 2:(half + 1) * 2],
                        in_=xbf_r[:, grp * 4 + half * 2:grp * 4 + half * 2 + 2])
                xn_all = w1.tile([128, 4, C], BF16, name="ln1_xn")
                tiles = [(xin[:, j], xn_all[:, j]) for j in range(4)]
                _ln_group(nc, w1, tiles, eps_sb)
                for j in range(4):
                    mt = grp * 4 + j
                    xn_ap = tiles[j][1]
                    tp = ps_t.tile([128, CB, 128], BF16, name="ln1_tp")
                    for cb in range(CB):
                        nc.tensor.transpose(
                            tp[:, cb, :], xn_ap[:, cb * 128:(cb + 1) * 128],
                            ident)
                    if j % 2 == 0:
                        nc.vector.tensor_copy(
                            out=hT[:, :, mt * 128:(mt + 1) * 128], in_=tp)
                    else:
                        nc.scalar.activation(
                            out=hT[:, :, mt * 128:(mt + 1) * 128], in_=tp,
                            func=AF.Copy)
        if STOP_AFTER == "ln1":
            return

        with (
            tc.tile_pool(name="qkvw", bufs=1) as qkvw,
            tc.tile_pool(name="p_kv", bufs=1) as p_kv,
            tc.tile_pool(name="qkv_ps", bufs=6, space="PSUM") as ps_q,
        ):
            # k projection, channel-major [c_k, t] fp8, full batch
            wk = qkvw.tile([128, CB, C], F8, name="wk")
            nc.sync.dma_start(out=wk, in_=qkv_r[:, :, C:2 * C])
            k_sb = p_kv.tile([128, CB, M], F8, name="k_sb")
            for kb in range(CB):
                for tch in range(M // T):
                    pk = ps_q.tile([128, T], FP32, name="pq")
                    for cb in range(0, CB, 2):
                        nc.tensor.matmul(
                            pk, wk[:, cb:cb + 2, kb * 128:(kb + 1) * 128],
                            hT[:, cb:cb + 2, tch * T:(tch + 1) * T],
                            start=(cb == 0), stop=(cb == CB - 2), perf_mode=DR)
                    nc.scalar.activation(
                        out=k_sb[:, kb, tch * T:(tch + 1) * T], in_=pk,
                        func=AF.Copy)
            # repack to the 32-partition DoubleRow layout: 4 shift DMAs
            for hh in range(2):
                for qq in range(2):
                    off = hh * 64 + qq * 32
                    nc.sync.dma_start(
                        out=kTd.rearrange("p (kb h q) t -> p h q kb t",
                                          h=2, q=2)[:, hh, qq],
                        in_=k_sb[off:off + 32, :, :])

            # v projection, token-major [t, h, d(+1)] into vg directly
            wv = qkvw.tile([128, CB, C], F8, name="wv")
            nc.sync.dma_start(out=wv, in_=qkv_r[:, :, 2 * C:3 * C])
            # one strided memset covers every tile's softmax-denominator
            # ones column (replaces 16 per-tile memsets and their sem edges)
            nc.vector.memset(vg[:, :, :, D:D + 1], 1.0)
            for mt in range(MT):
                for vc in range(2):
                    pv = ps_q.tile([128, T], FP32, name="pq")
                    for cb in range(0, CB, 2):
                        nc.tensor.matmul(
                            pv, hT[:, cb:cb + 2, mt * 128:(mt + 1) * 128],
                            wv[:, cb:cb + 2, vc * T:(vc + 1) * T],
                            start=(cb == 0), stop=(cb == CB - 2), perf_mode=DR)
                    nc.scalar.activation(
                        out=vg[:, mt, vc * 8:(vc + 1) * 8, 0:D],
                        in_=pv.rearrange("p (h d) -> p h d", d=D),
                        func=AF.Copy)

            # q projection (own tokens = first T columns of hT)
            wq = qkvw.tile([128, CB, C], F8, name="wq")
            nc.sync.dma_start(out=wq, in_=qkv_r[:, :, 0:C])
            qtmp = p_kv.tile([128, CB, T], F8, name="qtmp")
            for qb in range(CB):
                pq = ps_q.tile([128, T], FP32, name="pq")
                for cb in range(0, CB, 2):
                    nc.tensor.matmul(
                        pq, wq[:, cb:cb + 2, qb * 128:(qb + 1) * 128],
                        hT[:, cb:cb + 2, 0:T],
                        start=(cb == 0), stop=(cb == CB - 2), perf_mode=DR)
                nc.vector.tensor_scalar(
                    out=qtmp[:, qb, :], in0=pq,
                    scalar1=qb_sb[:, qb:qb + 1], scalar2=None,
                    op0=ALU.add)
            for hh in range(2):
                for qq in range(2):
                    off = hh * 64 + qq * 32
                    nc.sync.dma_start(
                        out=qTd[0:32, :, hh, qq, :],
                        in_=qtmp[off:off + 32, :, :])
        if STOP_AFTER == "qproj":
            return

    # -----------------------------------------------------------
    # Phase 3: attention pair loop with folded output projection
    # -----------------------------------------------------------
    # x2 starts as x + proj_b (residual base for the proj partials).
    # The fp32 own rows load directly into x2, off the phase-1/2 DMA path.
    b1bc = bcast_rows_pool(p_res, proj_b, "b1bc")
    nc.sync.dma_start(out=x2, in_=xown_r[:, 0:TB])
    for tb in range(TB):
        nc.vector.tensor_add(out=x2[:, tb, :], in0=x2[:, tb, :], in1=b1bc)

    with (
        tc.tile_pool(name="a_w", bufs=2) as pjw,
        tc.tile_pool(name="a_p", bufs=4) as pp,
        tc.tile_pool(name="a_r", bufs=4) as pr,
        tc.tile_pool(name="a_pss", bufs=2, space="PSUM") as ps_s,
        tc.tile_pool(name="a_pso", bufs=1, space="PSUM") as ps_o,
        tc.tile_pool(name="a_pspj", bufs=2, space="PSUM") as ps_pj,
    ):
        kslc = kTd.rearrange("p (pr h q) t -> p pr h q t", h=2, q=2)
        for pair in range(H // 2):
            oA = ps_o.tile([128, T], FP32, name="oA")   # rows 0:65
            oB = ps_o.tile([128, T], FP32, name="oB")
            pab = None
            for mi in range(MI):
                msl = slice(mi * 128, (mi + 1) * 128)
                sAB = ps_s.tile([128, 2, T], FP32, name="sAB")
                for hh in range(2):
                    nc.tensor.matmul(
                        sAB[:, hh, :], kslc[0:32, pair, hh, :, msl],
                        qTd[0:32, pair, hh], start=True, stop=True,
                        perf_mode=DR)
                if mi % 2 == 0:
                    pab = pp.tile([128, 2, 2, T], F8, name="pab")
                nc.scalar.activation(out=pab[:, mi % 2], in_=sAB,
                                     func=AF.Exp, scale=SCALE,
                                     bias=eshift_sb)
                if mi % 2 == 1:
                    # fp8 DoubleRow: both 128-key chunks in one MM per head
                    nc.tensor.matmul(
                        oA[0:D + 1, :], vg[:, mi - 1:mi + 1, 2 * pair, :],
                        pab[:, :, 0, :],
                        start=(mi == 1), stop=(mi == MI - 1), perf_mode=DR)
                    nc.tensor.matmul(
                        oB[0:D + 1, :], vg[:, mi - 1:mi + 1, 2 * pair + 1, :],
                        pab[:, :, 1, :],
                        start=(mi == 1), stop=(mi == MI - 1), perf_mode=DR)

            # normalize: oT[head] = o_unnorm * (1/sums); the per-token
            # reciprocal row is broadcast over the 64 head dims with an
            # ones-outer-product matmul. Head B's product is
            # partition-shifted to rows 64:128 with an SBUF->SBUF DMA.
            recb = pr.tile([128, T], BF16, name="recb")
            with nc.allow_low_precision("recip feeds fp8 oT; bf16 is plenty"):
                nc.vector.reciprocal(out=recb[64:65, :], in_=oA[64:65, :])
            rbA_ps = ps_pj.tile([128, 512], FP32, name="ppj")
            nc.tensor.matmul(
                rbA_ps[0:64, :], ones_b[64:65, 0:64],
                recb[64:65, :], start=True, stop=True)
            rbA = pr.tile([128, T], FP32, name="rbA")
            nc.vector.tensor_copy(out=rbA[0:64, :], in_=rbA_ps[0:64, :])
            rec2b = pr.tile([128, T], BF16, name="rec2b")
            with nc.allow_low_precision("recip feeds fp8 oT; bf16 is plenty"):
                nc.vector.reciprocal(out=rec2b[64:65, :], in_=oB[64:65, :])
            rbB_ps = ps_pj.tile([128, 512], FP32, name="ppj")
            nc.tensor.matmul(
                rbB_ps[0:64, :], ones_b[64:65, 0:64],
                rec2b[64:65, :], start=True, stop=True)
            rbB = pr.tile([128, T], FP32, name="rbB")
            nc.vector.tensor_copy(out=rbB[0:64, :], in_=rbB_ps[0:64, :])
            nc.vector.tensor_mul(
                out=oT[0:64, pair, :], in0=oA[0:64, :], in1=rbA[0:64, :])
            tmpB = pr.tile([128, T], F8, name="tmpB")
            nc.vector.tensor_mul(
                out=tmpB[0:64, :], in0=oB[0:64, :], in1=rbB[0:64, :])
            nc.sync.dma_start(
                out=oT[64:128, pair, :], in_=tmpB[0:64, :])

            # folded output projection, every second pair (fp8 DoubleRow
            # contracts both pairs' head dims at once): x2 += oT @ W
            if pair % 2 == 1:
                wpj = pjw.tile([128, 2, C], F8, name="wpj")
                nc.sync.dma_start(out=wpj,
                                  in_=proj_r[:, pair - 1:pair + 1, :])
                for tb in range(TB):
                    for cc in range(2):
                        ppj = ps_pj.tile([128, 512], FP32, name="ppj")
                        nc.tensor.matmul(
                            ppj, oT[:, pair - 1:pair + 1,
                                    tb * 128:(tb + 1) * 128],
                            wpj[:, :, cc * 512:(cc + 1) * 512],
                            start=True, stop=True, perf_mode=DR)
                        nc.vector.tensor_add(
                            out=x2[:, tb, cc * 512:(cc + 1) * 512],
                            in0=x2[:, tb, cc * 512:(cc + 1) * 512],
                            in1=ppj)

    attn_ctx.close()
    if STOP_AFTER == "attn":
        return
    # ---------------------------------------------------------------
    # Phase 4: LN2 -> h2T [c, t]; then x2 += fc2 bias (residual base)
    # ---------------------------------------------------------------
    with (
        tc.tile_pool(name="ln2_work", bufs=2) as w2,
        tc.tile_pool(name="ln2_ps", bufs=8, space="PSUM") as ps_t2,
    ):
        xn_all = w2.tile([128, TB, C], BF16, name="ln2_xn")
        tiles = [(x2[:, tb, :], xn_all[:, tb]) for tb in range(TB)]
        _ln_group(nc, w2, tiles, eps_sb)
        for tb in range(TB):
            xn_ap = tiles[tb][1]
            tp = ps_t2.tile([128, CB, 128], BF16, name="ln2_tp")
            for cb in range(CB):
                nc.tensor.transpose(
                    tp[:, cb, :], xn_ap[:, cb * 128:(cb + 1) * 128], ident)
            nc.vector.tensor_copy(
                out=h2T[:, :, tb * 128:(tb + 1) * 128], in_=tp)
    b2bc = bcast_rows_pool(p_res, fc2_b, "b2bc")
    for tb in range(TB):
        nc.vector.tensor_add(out=x2[:, tb, :], in0=x2[:, tb, :], in1=b2bc)

    if STOP_AFTER == "ln2":
        return
    # ---------------------------------------------------------------
    # Phase 5: MLP fc1 (gelu) -> gT [f, t]; fc2 + residual -> y
    # ---------------------------------------------------------------
    with (
        tc.tile_pool(name="p_g", bufs=1) as p_g,
        tc.tile_pool(name="f_w", bufs=3) as fw,
        tc.tile_pool(name="f_out", bufs=8) as fout,
    ):
        gT = p_g.tile([128, FB, T], BF16)

        with tc.tile_pool(name="f1_ps", bufs=6, space="PSUM") as ps_f1:
            for fq in range(FB // 4):
                w1t = fw.tile([128, CB, 512], BF16, name="w1t")
                nc.sync.dma_start(
                    out=w1t, in_=fc1_r[:, :, fq * 512:(fq + 1) * 512])
                for j in range(4):
                    fb = fq * 4 + j
                    pf = ps_f1.tile([128, T], FP32, name="pf")
                    for cb in range(CB):
                        nc.tensor.matmul(
                            pf, w1t[:, cb, j * 128:(j + 1) * 128],
                            h2T[:, cb, :],
                            start=(cb == 0), stop=(cb == CB - 1))
                    nc.scalar.activation(
                        out=gT[:, fb, :], in_=pf, func=AF.Gelu,
                        bias=fc1b_sb[:, fb:fb + 1], scale=1.0)

        # fc2: all 8 [t, c] psum accumulators live at once (8 banks), so
        # each weight tile streams exactly once.
        with tc.tile_pool(name="f2_ps", bufs=1, space="PSUM") as ps_f2:
            held = {}
            for tb in range(TB):
                for cc in range(2):
                    held[(tb, cc)] = ps_f2.tile(
                        [128, 512], FP32, name=f"pf2_{tb}_{cc}")
            for fg in range(FB // 4):
                w2t = fw.tile([128, 4, C], BF16, name="w2t")
                nc.sync.dma_start(out=w2t,
                                  in_=fc2_r[:, fg * 4:(fg + 1) * 4, :])
                for f4 in range(4):
                    fb = fg * 4 + f4
                    for tb in range(TB):
                        for cc in range(2):
                            nc.tensor.matmul(
                                held[(tb, cc)],
                                gT[:, fb, tb * 128:(tb + 1) * 128],
                                w2t[:, f4, cc * 512:(cc + 1) * 512],
                                start=(fb == 0), stop=(fb == FB - 1))
            for tb in range(TB):
                for cc in range(2):
                    yt = fout.tile([128, 512], FP32, name="yt")
                    nc.vector.tensor_add(
                        out=yt, in0=held[(tb, cc)],
                        in1=x2[:, tb, cc * 512:(cc + 1) * 512])
                    nc.sync.dma_start(
                        out=y_r[:, tb, cc * 512:(cc + 1) * 512], in_=yt)


def split_excess_waits(nc, limit=1):
    """This walrus build only supports ONE sync wait per engine instruction.
    Move excess waits onto NOPs inserted just before the instruction on the
    same engine (for DMAs, move all waits so the descriptor carries none)."""
    for f in nc.m.functions:
        for bb in f.blocks:
            new_insts = []
            for inst in bb.instructions:
                si = getattr(inst, "sync_info", None)
                if si is not None and si.on_wait and len(si.on_wait) > limit:
                    waits = list(si.on_wait)
                    if isinstance(inst, mybir.InstDMACopy):
                        moved, si.on_wait = waits, []
                    else:
                        moved, si.on_wait = waits[limit:], waits[:limit]
                    for j, w in enumerate(moved):
                        nop = mybir.InstNoOp(
                            name=f"{inst.name}-xw{j}",
                            engine=inst.engine,
                            sync_info=mybir.SyncInfo(on_wait=[w], on_update=[]),
                            bass_nofuse=True,
                        )
                        new_insts.append(nop)
                new_insts.append(inst)
            bb.instructions[:] = new_insts


_CACHE = {}


def build(repeat=1):
    key = (STOP_AFTER, repeat)
    if key in _CACHE:
        return _CACHE[key]

    nc = bass.Bass("TRN2", target_bir_lowering=False, debug=False,
                   num_devices=N_CORES)
    io = {}
    io["xfull"] = nc.dram_tensor("xfull", [M, C], FP32, kind="ExternalInput").ap()
    io["xbf"] = nc.dram_tensor("xbf", [M, C], BF16, kind="ExternalInput").ap()
    io["qkv_w"] = nc.dram_tensor("qkv_w", [C, 3 * C], F8, kind="ExternalInput").ap()
    io["proj_w"] = nc.dram_tensor("proj_w", [C, C], F8, kind="ExternalInput").ap()
    io["proj_b"] = nc.dram_tensor("proj_b", [C], FP32, kind="ExternalInput").ap()
    io["q_bias"] = nc.dram_tensor("q_bias", [C], FP32, kind="ExternalInput").ap()
    io["fc1_w"] = nc.dram_tensor("fc1_w", [C, F], BF16, kind="ExternalInput").ap()
    io["fc1_b"] = nc.dram_tensor("fc1_b", [F], FP32, kind="ExternalInput").ap()
    io["fc2_w"] = nc.dram_tensor("fc2_w", [F, C], BF16, kind="ExternalInput").ap()
    io["fc2_b"] = nc.dram_tensor("fc2_b", [C], FP32, kind="ExternalInput").ap()
    io["y"] = nc.dram_tensor("y", [T, C], FP32, kind="ExternalOutput").ap()

    with tile.TileContext(nc) as tc:
        for _rep in range(repeat):
            with ExitStack() as ctx:
                _emit(ctx, tc, io)

    split_excess_waits(nc)
    _CACHE[key] = nc
    return nc


def make_in_maps(inputs):
    x = np.ascontiguousarray(np.asarray(inputs["x"]), dtype=np.float32)
    f64 = {k: np.asarray(inputs[k], dtype=np.float64)
           for k in ("qkv_w", "proj_w", "proj_b", "ln1_w", "ln1_b", "ln2_w",
                     "ln2_b", "fc1_w", "fc1_b", "fc2_w", "fc2_b")}
    # Fold LayerNorm affines into the weights (exact up to fp32 rounding):
    #   h = xn*ln_w + ln_b;  h @ W = xn @ (ln_w[:,None]*W) + ln_b @ W
    # The k-part of the qkv bias cancels in softmax; the v-part commutes
    # through the (row-stochastic) attention matrix into proj_b.
    qkv_eff = f64["qkv_w"] * f64["ln1_w"][:, None]
    qkv_bias = f64["ln1_b"] @ f64["qkv_w"]        # [3072]
    q_bias = qkv_bias[0:C]
    v_bias = qkv_bias[2 * C:3 * C]
    proj_b_eff = f64["proj_b"] + v_bias @ f64["proj_w"]
    fc1_eff = f64["fc1_w"] * f64["ln2_w"][:, None]
    fc1_b_eff = f64["fc1_b"] + f64["ln2_b"] @ f64["fc1_w"]
    weights = {
        "qkv_w": qkv_eff, "q_bias": q_bias, "proj_w": f64["proj_w"],
        "proj_b": proj_b_eff, "fc1_w": fc1_eff, "fc1_b": fc1_b_eff,
        "fc2_w": f64["fc2_w"], "fc2_b": f64["fc2_b"],
    }
    weights = {k: np.ascontiguousarray(v, dtype=np.float32)
               for k, v in weights.items()}
    import ml_dtypes
    for k in ("qkv_w", "proj_w"):
        weights[k] = weights[k].astype(ml_dtypes.float8_e4m3)
    for k in ("fc1_w", "fc2_w"):
        weights[k] = weights[k].astype(ml_dtypes.bfloat16)
    maps = []
    for c in range(N_CORES):
        b, q = c // 4, c % 4
        m = dict(weights)
        xb = x[b]
        xf = np.ascontiguousarray(np.concatenate(
            [xb[q * T:(q + 1) * T], xb[:q * T], xb[(q + 1) * T:]], axis=0))
        m["xfull"] = xf
        m["xbf"] = xf.astype(ml_dtypes.bfloat16)
        maps.append(m)
    return maps


def assemble(results):
    out = np.empty((B, N, C), dtype=np.float32)
    for c in range(N_CORES):
        b, q = c // 4, c % 4
        out[b, q * T:(q + 1) * T] = results[c]["y"]
    return out


def kernel(**inputs) -> np.ndarray:
    nc = build()
    res = run_bass_kernel_spmd(nc, make_in_maps(inputs), list(range(N_CORES)))
    return assemble(res.results)
